# revision 11
# baseline (speedup 1.0000x reference)
"""Trainium2 Bass kernel for nn_CapsuleLayer (capsule dynamic routing).

Math (reference):
    u_hat[b,c,u,s] = sum_i W[c,u,s,i] * x[b,i,c]          (never materialized here)
    3 routing iterations:
        c_ij = softmax_u(b_ij)                            [C, U]
        s_j[b,u,s]  = sum_c c_ij[c,u] * u_hat[b,c,u,s]
        v_j = squash(s_j)   (norm over the U axis!)
        u_vj1[c,u] = sum_{b,s} u_hat[b,c,u,s] v_j[b,u,s] / B
        b_ij += u_vj1
    output = v_j  (B, U, S, 1)

Sharding: channels C=1152 split 8 ways (CL=144 per core).  Per core both
x-slice and W-slice live in SBUF, so u_hat is recomputed on the fly as
matrix products (contraction over (c,i)=2304 or over b=256), all shapes
128-partition friendly.  The only cross-core data dependency is the
s_j partial sum: one 320 KB AllReduce per routing iteration.

Per-core layouts (host-prepared, all f32):
    XT [128, T*B ]  : XT[p, t*256+b]      = x[b, i, c],  ci = 128t+p
    XF [128, 2*KCI] : XF[p, bc*2304+ci]   = x[b, i, c],  b  = 128bc+p
    WM [128, T*US]  : WM[p, t*320+s*10+u] = W[c, u, s, i], ci = 128t+p
    EM [128, 128]   : block-diag selector, EM[p,m] = (p//16==m//16)/256
Column convention for the (u,s) axis everywhere: col = s*10 + u.
"""

import numpy as np

B, IN_U, C, NUM_U, S = 256, 16, 1152, 10, 32
NCORES = 8
CL = C // NCORES          # 144 channels per core
KCI = CL * IN_U           # 2304 contraction size
T = KCI // 128            # 18 partition chunks
US = NUM_U * S            # 320
NITER = 3

_CACHE = {}


def _build_program():
    import concourse.bacc as bacc
    import concourse.tile as tile
    from concourse import mybir
    from contextlib import ExitStack

    f32 = mybir.dt.float32
    AX = mybir.AxisListType
    ALU = mybir.AluOpType
    AF = mybir.ActivationFunctionType

    # Bacc (not raw Bass): its compile() pipeline legalizes multi-wait
    # instructions (move_matmul_waits_to_ldweights + generate_event_semaphores)
    # which walrus codegen otherwise rejects ("Too many sync wait commands").
    nc = bacc.Bacc(None, num_devices=NCORES)
    xt_d = nc.declare_dram_parameter("xt", [128, T * B], f32, isOutput=False)
    xf_d = nc.declare_dram_parameter("xf", [128, 2 * KCI], f32, isOutput=False)
    wm_d = nc.declare_dram_parameter("wm", [128, T * US], f32, isOutput=False)
    em_d = nc.declare_dram_parameter("em", [128, 128], f32, isOutput=False)
    out_d = nc.declare_dram_parameter("out", [128, 2 * US], f32, isOutput=True)

    with tile.TileContext(nc) as tc, ExitStack() as ctx:
        singles = ctx.enter_context(tc.tile_pool(name="singles", bufs=1))
        big = ctx.enter_context(tc.tile_pool(name="big", bufs=1))
        work = ctx.enter_context(tc.tile_pool(name="work", bufs=2))
        psum_s = ctx.enter_context(tc.tile_pool(name="psum_s", bufs=2, space="PSUM"))
        psum_m = ctx.enter_context(tc.tile_pool(name="psum_m", bufs=4, space="PSUM"))
        psum_u = ctx.enter_context(tc.tile_pool(name="psum_u", bufs=1, space="PSUM"))
        dram = ctx.enter_context(tc.tile_pool(name="dram", bufs=2, space="DRAM"))

        xt_sb = singles.tile([128, T * B], f32, name="xt_sb")
        nc.sync.dma_start(out=xt_sb, in_=xt_d[:])
        wm_sb = singles.tile([128, T * US], f32, name="wm_sb")
        nc.sync.dma_start(out=wm_sb, in_=wm_d[:])
        xf_sb = singles.tile([128, 2 * KCI], f32, name="xf_sb")
        nc.sync.dma_start(out=xf_sb, in_=xf_d[:])
        em_sb = singles.tile([128, 128], f32, name="em_sb")
        nc.sync.dma_start(out=em_sb, in_=em_d[:])
        bij_sb = singles.tile([128, T * NUM_U], f32, name="bij_sb")

        def mm1(rhs_sb, scale):
            """s_partial[b,(s,u)] = XT.T @ rhs, scaled; -> [128, 2*US] sbuf."""
            cc_sb = work.tile([128, 2 * US], f32, name="cc_sb")
            for bc in range(2):
                ps = psum_s.tile([128, US], f32, name="s_ps")
                for t in range(T):
                    nc.tensor.matmul(
                        ps,
                        lhsT=xt_sb[:, t * B + bc * 128 : t * B + bc * 128 + 128],
                        rhs=rhs_sb[:, t * US : (t + 1) * US],
                        start=(t == 0),
                        stop=(t == T - 1),
                    )
                nc.scalar.activation(
                    out=cc_sb[:, bc * US : (bc + 1) * US],
                    in_=ps,
                    func=AF.Copy,
                    scale=float(scale),
                )
            return cc_sb

        def allreduce(cc_sb):
            cc_in = dram.tile([128, 2 * US], f32, name="cc_in")
            cc_out = dram.tile([128, 2 * US], f32, name="cc_out")
            nc.gpsimd.dma_start(out=cc_in, in_=cc_sb)
            nc.gpsimd.collective_compute(
                "AllReduce",
                ALU.add,
                replica_groups=[list(range(NCORES))],
                ins=[cc_in.opt()],
                outs=[cc_out.opt()],
            )
            s_sb = work.tile([128, 2 * US], f32, name="s_sb")
            nc.gpsimd.dma_start(out=s_sb, in_=cc_out)
            return s_sb

        def squash(s_sb):
            """v = |s|^2/(1+|s|^2) * s/|s|, norm taken over u per (b, s')."""
            sq = work.tile([128, 2 * US], f32, name="sq")
            nc.scalar.square(out=sq, in_=s_sb)
            magsq = work.tile([128, 2 * S], f32, name="magsq")
            nc.vector.reduce_sum(
                out=magsq,
                in_=sq.rearrange("p (bc s u) -> p bc s u", bc=2, s=S),
                axis=AX.X,
            )
            mag = work.tile([128, 2 * S], f32, name="mag")
            nc.scalar.sqrt(out=mag, in_=magsq)
            den = work.tile([128, 2 * S], f32, name="den")
            nc.vector.scalar_tensor_tensor(
                out=den, in0=magsq, scalar=1.0, in1=mag, op0=ALU.add, op1=ALU.mult
            )
            rden = work.tile([128, 2 * S], f32, name="rden")
            nc.vector.reciprocal(out=rden, in_=den)
            fct = work.tile([128, 2 * S], f32, name="fct")
            nc.vector.tensor_mul(out=fct, in0=magsq, in1=rden)
            v_sb = work.tile([128, 2 * US], f32, name="v_sb")
            nc.vector.tensor_mul(
                out=v_sb.rearrange("p (bc s u) -> p bc s u", bc=2, s=S),
                in0=s_sb.rearrange("p (bc s u) -> p bc s u", bc=2, s=S),
                in1=fct.rearrange("p (bc s) -> p bc s", bc=2)
                .unsqueeze(3)
                .broadcast_to([128, 2, S, NUM_U]),
            )
            return v_sb

        def routing_update(v_sb, first):
            """u_vj1 -> b_ij update -> softmax; returns c_ij [128, T*NUM_U]."""
            m_sb = big.tile([128, T * US], f32, name="m_sb")
            for t in range(T):
                ps = psum_m.tile([128, US], f32, name="m_ps")
                for bc in range(2):
                    nc.tensor.matmul(
                        ps,
                        lhsT=xf_sb[:, bc * KCI + t * 128 : bc * KCI + (t + 1) * 128],
                        rhs=v_sb[:, bc * US : (bc + 1) * US],
                        start=(bc == 0),
                        stop=(bc == 1),
                    )
                nc.scalar.copy(out=m_sb[:, t * US : (t + 1) * US], in_=ps)
            q_sb = big.tile([128, T * US], f32, name="q_sb")
            nc.vector.tensor_mul(out=q_sb, in0=wm_sb, in1=m_sb)
            r_sb = work.tile([128, T * NUM_U], f32, name="r_sb")
            nc.vector.reduce_sum(
                out=r_sb,
                in_=q_sb.rearrange("p (t s u) -> p t s u", t=T, s=S).transpose(
                    [0, 1, 3, 2]
                ),
                axis=AX.X,
            )
            ups = psum_u.tile([128, T * NUM_U], f32, name="u_ps")
            nc.tensor.matmul(ups, lhsT=em_sb, rhs=r_sb, start=True, stop=True)
            if first:
                nc.vector.tensor_copy(out=bij_sb, in_=ups)
            else:
                nc.vector.tensor_add(out=bij_sb, in0=bij_sb, in1=ups)
            # softmax over u (inner groups of 10)
            mx = work.tile([128, T], f32, name="mx")
            nc.vector.reduce_max(
                out=mx, in_=bij_sb.rearrange("p (t u) -> p t u", t=T), axis=AX.X
            )
            ex = work.tile([128, T * NUM_U], f32, name="ex")
            nc.vector.tensor_sub(
                out=ex.rearrange("p (t u) -> p t u", t=T),
                in0=bij_sb.rearrange("p (t u) -> p t u", t=T),
                in1=mx[:].unsqueeze(2).broadcast_to([128, T, NUM_U]),
            )
            nc.scalar.activation(out=ex, in_=ex, func=AF.Exp)
            sm = work.tile([128, T], f32, name="sm")
            nc.vector.reduce_sum(
                out=sm, in_=ex.rearrange("p (t u) -> p t u", t=T), axis=AX.X
            )
            rsm = work.tile([128, T], f32, name="rsm")
            nc.vector.reciprocal(out=rsm, in_=sm)
            cij_sb = work.tile([128, T * NUM_U], f32, name="cij_sb")
            nc.vector.tensor_mul(
                out=cij_sb.rearrange("p (t u) -> p t u", t=T),
                in0=ex.rearrange("p (t u) -> p t u", t=T),
                in1=rsm[:].unsqueeze(2).broadcast_to([128, T, NUM_U]),
            )
            return cij_sb

        def bm_build(cij_sb):
            bm_sb = big.tile([128, T * US], f32, name="bm_sb")
            nc.vector.tensor_mul(
                out=bm_sb.rearrange("p (t s u) -> p t s u", t=T, s=S),
                in0=wm_sb.rearrange("p (t s u) -> p t s u", t=T, s=S),
                in1=cij_sb.rearrange("p (t u) -> p t u", t=T)
                .unsqueeze(2)
                .broadcast_to([128, T, S, NUM_U]),
            )
            return bm_sb

        v_sb = None
        for it in range(NITER):
            if it == 0:
                cc = mm1(wm_sb, 1.0 / NUM_U)
            else:
                cij = routing_update(v_sb, first=(it == 1))
                cc = mm1(bm_build(cij), 1.0)
            s_sb = allreduce(cc)
            v_sb = squash(s_sb)
        nc.sync.dma_start(out=out_d[:], in_=v_sb)

    return nc


def _prep_core_inputs(x, W, core, em):
    sl = slice(core * CL, (core + 1) * CL)
    xs = np.ascontiguousarray(x[:, :, sl])  # (B, I, CL)
    ws = np.ascontiguousarray(W[0, sl])     # (CL, U, S, I)
    xt = xs.transpose(2, 1, 0).reshape(T, 128, B)
    xt = np.ascontiguousarray(xt.transpose(1, 0, 2)).reshape(128, T * B)
    xf = xs.transpose(0, 2, 1).reshape(2, 128, KCI)
    xf = np.ascontiguousarray(xf.transpose(1, 0, 2)).reshape(128, 2 * KCI)
    wm = ws.transpose(0, 3, 2, 1).reshape(T, 128, US)
    wm = np.ascontiguousarray(wm.transpose(1, 0, 2)).reshape(128, T * US)
    return {"xt": xt, "xf": xf, "wm": wm, "em": em}


def prep_in_maps(x, W):
    x = np.asarray(x, dtype=np.float32)
    W = np.asarray(W, dtype=np.float32)
    em = (np.kron(np.eye(8, dtype=np.float32), np.ones((16, 16), np.float32))
          / float(B))
    return [_prep_core_inputs(x, W, core, em) for core in range(NCORES)]


def postprocess(out_core):
    """out_core [128, 640] with col = bc*320 + s*10 + u -> (B, U, S, 1)."""
    v = out_core.reshape(128, 2, S, NUM_U).transpose(1, 0, 3, 2)  # (bc,p,u,s)
    return np.ascontiguousarray(v.reshape(B, NUM_U, S)[..., None])


def get_program():
    if "nc" not in _CACHE:
        nc = _build_program()
        nc.finalize()  # runs Bacc.compile(): reg alloc + sync-wait legalization
        _CACHE["nc"] = nc
    return _CACHE["nc"]


def kernel(x, W):
    from concourse.bass_utils import run_bass_kernel_spmd

    nc = get_program()
    in_maps = prep_in_maps(x, W)
    res = run_bass_kernel_spmd(nc, in_maps, list(range(NCORES)))
    return postprocess(np.asarray(res.results[0]["out"], dtype=np.float32))


# revision 16
# speedup vs baseline: 1.5691x; 1.5691x over previous
"""Trainium2 Bass kernel for nn_CapsuleLayer (capsule dynamic routing).

Math (reference):
    u_hat[b,c,u,s] = sum_i W[c,u,s,i] * x[b,i,c]          (never materialized here)
    3 routing iterations:
        c_ij = softmax_u(b_ij)                            [C, U]
        s_j[b,u,s]  = sum_c c_ij[c,u] * u_hat[b,c,u,s]
        v_j = squash(s_j)   (norm over the U axis!)
        u_vj1[c,u] = sum_{b,s} u_hat[b,c,u,s] v_j[b,u,s] / B
        b_ij += u_vj1
    output = v_j  (B, U, S, 1)

Sharding: channels C=1152 split 8 ways (CL=144 per core).  Per core both
x-slice and W-slice live in SBUF, so u_hat is recomputed on the fly as
matrix products (contraction over (c,i)=2304 or over b=256), all shapes
128-partition friendly.  The only cross-core data dependency is the
s_j partial sum: one 320 KB AllReduce per routing iteration.

Inputs x/W are staged in f16 (matmul + elementwise operands); all
accumulations (PSUM), the AllReduce, squash and softmax stay fp32.

Per-core layouts (host-prepared):
    XT [128, T*B ] f16 : XT[p, t*256+b]      = x[b, i, c],  ci = 128t+p
    XF [128, 2*KCI] f16: XF[p, bc*2304+ci]   = x[b, i, c],  b  = 128bc+p
    WM [128, T*US] f16 : WM[p, t*320+s*10+u] = W[c, u, s, i], ci = 128t+p
    EM [128, 128] f32   : block-diag selector, EM[p,m] = (p//16==m//16)/256
Column convention for the (u,s) axis everywhere: col = s*10 + u.
"""

import numpy as np

B, IN_U, C, NUM_U, S = 256, 16, 1152, 10, 32
NCORES = 8
CL = C // NCORES          # 144 channels per core
KCI = CL * IN_U           # 2304 contraction size
T = KCI // 128            # 18 partition chunks
US = NUM_U * S            # 320
NITER = 3
G = 3                     # chunk groups (pipelining granularity)
CPG = T // G              # 6 chunks per group

_CACHE = {}


def _build_program(bypass_cc=False):
    import concourse.bacc as bacc
    import concourse.tile as tile
    from concourse import mybir
    from contextlib import ExitStack

    f32 = mybir.dt.float32
    f16 = mybir.dt.float16
    AX = mybir.AxisListType
    ALU = mybir.AluOpType
    AF = mybir.ActivationFunctionType

    # Bacc (not raw Bass): its compile() pipeline legalizes multi-wait
    # instructions (move_matmul_waits_to_ldweights + generate_event_semaphores)
    # which walrus codegen otherwise rejects ("Too many sync wait commands").
    nc = bacc.Bacc(None, num_devices=NCORES)
    xt_d = nc.declare_dram_parameter("xt", [128, T * B], f16, isOutput=False)
    xf_d = nc.declare_dram_parameter("xf", [128, 2 * KCI], f16, isOutput=False)
    wm_d = nc.declare_dram_parameter("wm", [128, T * US], f16, isOutput=False)
    em_d = nc.declare_dram_parameter("em", [128, 128], f32, isOutput=False)
    out_d = nc.declare_dram_parameter("out", [128, 2 * US], f32, isOutput=True)

    with tile.TileContext(nc) as tc, ExitStack() as ctx:
        singles = ctx.enter_context(tc.tile_pool(name="singles", bufs=1))
        big = ctx.enter_context(tc.tile_pool(name="big", bufs=1))
        work = ctx.enter_context(tc.tile_pool(name="work", bufs=2))
        psum_s = ctx.enter_context(tc.tile_pool(name="psum_s", bufs=1, space="PSUM"))
        psum_m = ctx.enter_context(tc.tile_pool(name="psum_m", bufs=4, space="PSUM"))
        psum_u = ctx.enter_context(tc.tile_pool(name="psum_u", bufs=1, space="PSUM"))
        dram = ctx.enter_context(tc.tile_pool(name="dram", bufs=2, space="DRAM"))

        # Loads, grouped so iteration-1 MM1 can start after the first group.
        xt_sb = singles.tile([128, T * B], f16, name="xt_sb")
        wm_sb = singles.tile([128, T * US], f16, name="wm_sb")
        for g in range(G):
            nc.sync.dma_start(
                out=xt_sb[:, g * CPG * B : (g + 1) * CPG * B],
                in_=xt_d[:, g * CPG * B : (g + 1) * CPG * B],
            )
            nc.sync.dma_start(
                out=wm_sb[:, g * CPG * US : (g + 1) * CPG * US],
                in_=wm_d[:, g * CPG * US : (g + 1) * CPG * US],
            )
        xf_sb = singles.tile([128, 2 * KCI], f16, name="xf_sb")
        for bc in range(2):
            nc.sync.dma_start(
                out=xf_sb[:, bc * KCI : (bc + 1) * KCI],
                in_=xf_d[:, bc * KCI : (bc + 1) * KCI],
            )
        em_sb = singles.tile([128, 128], f32, name="em_sb")
        nc.sync.dma_start(out=em_sb, in_=em_d[:])
        bij_sb = singles.tile([128, T * NUM_U], f32, name="bij_sb")

        def mm1(rhs_groups, scale):
            """s_partial[b,(s,u)] = XT.T @ rhs, scaled; -> [128, 2*US] f32."""
            cc_sb = work.tile([128, 2 * US], f32, name="cc_sb")
            ps = [psum_s.tile([128, US], f32, name=f"s_ps{bc}") for bc in range(2)]
            for t in range(T):
                rhs = rhs_groups[t // CPG]
                tl = t % CPG
                for bc in range(2):
                    nc.tensor.matmul(
                        ps[bc],
                        lhsT=xt_sb[:, t * B + bc * 128 : t * B + bc * 128 + 128],
                        rhs=rhs[:, tl * US : (tl + 1) * US],
                        start=(t == 0),
                        stop=(t == T - 1),
                    )
            for bc in range(2):
                nc.scalar.activation(
                    out=cc_sb[:, bc * US : (bc + 1) * US],
                    in_=ps[bc],
                    func=AF.Copy,
                    scale=float(scale),
                )
            return cc_sb

        def allreduce(cc_sb):
            cc_in = dram.tile([128, 2 * US], f32, name="cc_in")
            cc_out = dram.tile([128, 2 * US], f32, name="cc_out")
            nc.gpsimd.dma_start(out=cc_in, in_=cc_sb)
            if bypass_cc:
                nc.gpsimd.dma_start(out=cc_out, in_=cc_in)
            else:
                nc.gpsimd.collective_compute(
                    "AllReduce",
                    ALU.add,
                    replica_groups=[list(range(NCORES))],
                    ins=[cc_in.opt()],
                    outs=[cc_out.opt()],
                )
            s_sb = work.tile([128, 2 * US], f32, name="s_sb")
            nc.gpsimd.dma_start(out=s_sb, in_=cc_out)
            return s_sb

        def squash(s_sb, want_bf):
            """v = |s|^2/(1+|s|^2) * s/|s|, norm over u per (b, s')."""
            sq = work.tile([128, 2 * US], f32, name="sq")
            nc.scalar.square(out=sq, in_=s_sb)
            magsq = work.tile([128, 2 * S], f32, name="magsq")
            nc.vector.reduce_sum(
                out=magsq,
                in_=sq.rearrange("p (bc s u) -> p bc s u", bc=2, s=S),
                axis=AX.X,
            )
            mag = work.tile([128, 2 * S], f32, name="mag")
            nc.scalar.sqrt(out=mag, in_=magsq)
            den = work.tile([128, 2 * S], f32, name="den")
            nc.vector.scalar_tensor_tensor(
                out=den, in0=magsq, scalar=1.0, in1=mag, op0=ALU.add, op1=ALU.mult
            )
            rden = work.tile([128, 2 * S], f32, name="rden")
            nc.vector.reciprocal(out=rden, in_=den)
            fct = work.tile([128, 2 * S], f32, name="fct")
            nc.vector.tensor_mul(out=fct, in0=magsq, in1=rden)
            v_sb = work.tile([128, 2 * US], f32, name="v_sb")
            nc.vector.tensor_mul(
                out=v_sb.rearrange("p (bc s u) -> p bc s u", bc=2, s=S),
                in0=s_sb.rearrange("p (bc s u) -> p bc s u", bc=2, s=S),
                in1=fct.rearrange("p (bc s) -> p bc s", bc=2)
                .unsqueeze(3)
                .broadcast_to([128, 2, S, NUM_U]),
            )
            if not want_bf:
                return v_sb, None
            v_bf = work.tile([128, 2 * US], f16, name="v_bf")
            nc.scalar.copy(out=v_bf, in_=v_sb)
            return v_sb, v_bf

        def routing_update(v_bf, first):
            """u_vj1 -> b_ij update -> softmax; returns c_ij (f16)."""
            r_sb = work.tile([128, T * NUM_U], f32, name="r_sb")
            for g in range(G):
                m_g = big.tile([128, CPG * US], f16, name=f"m_g{g}")
                for tl in range(CPG):
                    t = g * CPG + tl
                    ps = psum_m.tile([128, US], f32, name="m_ps")
                    for bc in range(2):
                        nc.tensor.matmul(
                            ps,
                            lhsT=xf_sb[
                                :, bc * KCI + t * 128 : bc * KCI + (t + 1) * 128
                            ],
                            rhs=v_bf[:, bc * US : (bc + 1) * US],
                            start=(bc == 0),
                            stop=(bc == 1),
                        )
                    nc.scalar.copy(out=m_g[:, tl * US : (tl + 1) * US], in_=ps)
                q_g = big.tile([128, CPG * US], f16, name=f"q_g{g}")
                nc.vector.tensor_mul(
                    out=q_g,
                    in0=wm_sb[:, g * CPG * US : (g + 1) * CPG * US],
                    in1=m_g,
                )
                nc.vector.reduce_sum(
                    out=r_sb[:, g * CPG * NUM_U : (g + 1) * CPG * NUM_U],
                    in_=q_g.rearrange("p (t s u) -> p t s u", t=CPG, s=S).transpose(
                        [0, 1, 3, 2]
                    ),
                    axis=AX.X,
                )
            ups = psum_u.tile([128, T * NUM_U], f32, name="u_ps")
            nc.tensor.matmul(ups, lhsT=em_sb, rhs=r_sb, start=True, stop=True)
            if first:
                nc.vector.tensor_copy(out=bij_sb, in_=ups)
            else:
                nc.vector.tensor_add(out=bij_sb, in0=bij_sb, in1=ups)
            # softmax over u (inner groups of 10)
            mx = work.tile([128, T], f32, name="mx")
            nc.vector.reduce_max(
                out=mx, in_=bij_sb.rearrange("p (t u) -> p t u", t=T), axis=AX.X
            )
            ex = work.tile([128, T * NUM_U], f32, name="ex")
            nc.vector.tensor_sub(
                out=ex.rearrange("p (t u) -> p t u", t=T),
                in0=bij_sb.rearrange("p (t u) -> p t u", t=T),
                in1=mx[:].unsqueeze(2).broadcast_to([128, T, NUM_U]),
            )
            nc.scalar.activation(out=ex, in_=ex, func=AF.Exp)
            sm = work.tile([128, T], f32, name="sm")
            nc.vector.reduce_sum(
                out=sm, in_=ex.rearrange("p (t u) -> p t u", t=T), axis=AX.X
            )
            rsm = work.tile([128, T], f32, name="rsm")
            nc.vector.reciprocal(out=rsm, in_=sm)
            cij_sb = work.tile([128, T * NUM_U], f16, name="cij_sb")
            nc.vector.tensor_mul(
                out=cij_sb.rearrange("p (t u) -> p t u", t=T),
                in0=ex.rearrange("p (t u) -> p t u", t=T),
                in1=rsm[:].unsqueeze(2).broadcast_to([128, T, NUM_U]),
            )
            return cij_sb

        def bm_build(cij_sb):
            groups = []
            for g in range(G):
                bm_g = big.tile([128, CPG * US], f16, name=f"bm_g{g}")
                nc.vector.tensor_mul(
                    out=bm_g.rearrange("p (t s u) -> p t s u", t=CPG, s=S),
                    in0=wm_sb[:, g * CPG * US : (g + 1) * CPG * US].rearrange(
                        "p (t s u) -> p t s u", t=CPG, s=S
                    ),
                    in1=cij_sb[:, g * CPG * NUM_U : (g + 1) * CPG * NUM_U]
                    .rearrange("p (t u) -> p t u", t=CPG)
                    .unsqueeze(2)
                    .broadcast_to([128, CPG, S, NUM_U]),
                )
                groups.append(bm_g)
            return groups

        wm_groups = [
            wm_sb[:, g * CPG * US : (g + 1) * CPG * US] for g in range(G)
        ]
        v_sb = None
        for it in range(NITER):
            if it == 0:
                cc = mm1(wm_groups, 1.0 / NUM_U)
            else:
                cij = routing_update(v_bf, first=(it == 1))
                cc = mm1(bm_build(cij), 1.0)
            s_sb = allreduce(cc)
            v_sb, v_bf = squash(s_sb, want_bf=(it < NITER - 1))
        nc.sync.dma_start(out=out_d[:], in_=v_sb)

    return nc


def _prep_core_inputs(x, W, core, em):
    sl = slice(core * CL, (core + 1) * CL)
    xs = np.ascontiguousarray(x[:, :, sl])  # (B, I, CL)
    ws = np.ascontiguousarray(W[0, sl])     # (CL, U, S, I)
    xt = xs.transpose(2, 1, 0).reshape(T, 128, B)
    xt = np.ascontiguousarray(xt.transpose(1, 0, 2)).reshape(128, T * B)
    xf = xs.transpose(0, 2, 1).reshape(2, 128, KCI)
    xf = np.ascontiguousarray(xf.transpose(1, 0, 2)).reshape(128, 2 * KCI)
    wm = ws.transpose(0, 3, 2, 1).reshape(T, 128, US)
    wm = np.ascontiguousarray(wm.transpose(1, 0, 2)).reshape(128, T * US)
    return {
        "xt": xt.astype(np.float16),
        "xf": xf.astype(np.float16),
        "wm": wm.astype(np.float16),
        "em": em,
    }


def prep_in_maps(x, W):
    x = np.asarray(x, dtype=np.float32)
    W = np.asarray(W, dtype=np.float32)
    em = (np.kron(np.eye(8, dtype=np.float32), np.ones((16, 16), np.float32))
          / float(B))
    return [_prep_core_inputs(x, W, core, em) for core in range(NCORES)]


def postprocess(out_core):
    """out_core [128, 640] with col = bc*320 + s*10 + u -> (B, U, S, 1)."""
    v = out_core.reshape(128, 2, S, NUM_U).transpose(1, 0, 3, 2)  # (bc,p,u,s)
    return np.ascontiguousarray(v.reshape(B, NUM_U, S)[..., None])


def get_program():
    if "nc" not in _CACHE:
        nc = _build_program()
        nc.finalize()  # runs Bacc.compile(): reg alloc + sync-wait legalization
        _CACHE["nc"] = nc
    return _CACHE["nc"]


def kernel(x, W):
    from concourse.bass_utils import run_bass_kernel_spmd

    nc = get_program()
    in_maps = prep_in_maps(x, W)
    res = run_bass_kernel_spmd(nc, in_maps, list(range(NCORES)))
    return postprocess(np.asarray(res.results[0]["out"], dtype=np.float32))


# revision 17
# speedup vs baseline: 5.2316x; 3.3342x over previous
"""Trainium2 Bass kernel for nn_CapsuleLayer (capsule dynamic routing).

Math (reference):
    u_hat[b,c,u,s] = sum_i W[c,u,s,i] * x[b,i,c]          (never materialized here)
    3 routing iterations:
        c_ij = softmax_u(b_ij)                            [C, U]
        s_j[b,u,s]  = sum_c c_ij[c,u] * u_hat[b,c,u,s]
        v_j = squash(s_j)   (norm over the U axis!)
        u_vj1[c,u] = sum_{b,s} u_hat[b,c,u,s] v_j[b,u,s] / B
        b_ij += u_vj1
    output = v_j  (B, U, S, 1)

Sharding: channels C=1152 split 8 ways (CL=144 per core).  Per core both
x-slice and W-slice live in SBUF, so u_hat is recomputed on the fly as
matrix products (contraction over (c,i)=2304 or over b=256), all shapes
128-partition friendly.  The only cross-core data dependency is the
s_j partial sum: one 320 KB AllReduce per routing iteration.

Inputs x/W are staged in f16 (matmul + elementwise operands); all
accumulations (PSUM), the AllReduce, squash and softmax stay fp32.

Per-core layouts (host-prepared):
    XT [128, T*B ] f16 : XT[p, t*256+b]      = x[b, i, c],  ci = 128t+p
    XF [128, 2*KCI] f16: XF[p, bc*2304+ci]   = x[b, i, c],  b  = 128bc+p
    WM [128, T*US] f16 : WM[p, t*320+s*10+u] = W[c, u, s, i], ci = 128t+p
    EM [128, 128] f32   : block-diag selector, EM[p,m] = (p//16==m//16)/256
Column convention for the (u,s) axis everywhere: col = s*10 + u.
"""

import numpy as np

B, IN_U, C, NUM_U, S = 256, 16, 1152, 10, 32
NCORES = 8
CL = C // NCORES          # 144 channels per core
KCI = CL * IN_U           # 2304 contraction size
T = KCI // 128            # 18 partition chunks
US = NUM_U * S            # 320
NITER = 3
G = 3                     # chunk groups (pipelining granularity)
CPG = T // G              # 6 chunks per group

_CACHE = {}


def _build_program(bypass_cc=False):
    import concourse.bacc as bacc
    import concourse.tile as tile
    from concourse import mybir
    from contextlib import ExitStack

    f32 = mybir.dt.float32
    f16 = mybir.dt.float16
    AX = mybir.AxisListType
    ALU = mybir.AluOpType
    AF = mybir.ActivationFunctionType

    # Bacc (not raw Bass): its compile() pipeline legalizes multi-wait
    # instructions (move_matmul_waits_to_ldweights + generate_event_semaphores)
    # which walrus codegen otherwise rejects ("Too many sync wait commands").
    nc = bacc.Bacc(None, num_devices=NCORES)
    xt_d = nc.declare_dram_parameter("xt", [128, T * B], f16, isOutput=False)
    xf_d = nc.declare_dram_parameter("xf", [128, 2 * KCI], f16, isOutput=False)
    wm_d = nc.declare_dram_parameter("wm", [128, T * US], f16, isOutput=False)
    em_d = nc.declare_dram_parameter("em", [128, 128], f32, isOutput=False)
    out_d = nc.declare_dram_parameter("out", [128, 2 * US], f32, isOutput=True)

    with tile.TileContext(nc) as tc, ExitStack() as ctx:
        singles = ctx.enter_context(tc.tile_pool(name="singles", bufs=1))
        big = ctx.enter_context(tc.tile_pool(name="big", bufs=1))
        work = ctx.enter_context(tc.tile_pool(name="work", bufs=2))
        psum_s = ctx.enter_context(tc.tile_pool(name="psum_s", bufs=1, space="PSUM"))
        psum_m = ctx.enter_context(tc.tile_pool(name="psum_m", bufs=4, space="PSUM"))
        psum_u = ctx.enter_context(tc.tile_pool(name="psum_u", bufs=1, space="PSUM"))
        dram = ctx.enter_context(tc.tile_pool(name="dram", bufs=2, space="DRAM"))

        # Loads, grouped so iteration-1 MM1 can start after the first group.
        xt_sb = singles.tile([128, T * B], f16, name="xt_sb")
        wm_sb = singles.tile([128, T * US], f16, name="wm_sb")
        for g in range(G):
            nc.sync.dma_start(
                out=xt_sb[:, g * CPG * B : (g + 1) * CPG * B],
                in_=xt_d[:, g * CPG * B : (g + 1) * CPG * B],
            )
            nc.sync.dma_start(
                out=wm_sb[:, g * CPG * US : (g + 1) * CPG * US],
                in_=wm_d[:, g * CPG * US : (g + 1) * CPG * US],
            )
        xf_sb = singles.tile([128, 2 * KCI], f16, name="xf_sb")
        for bc in range(2):
            nc.sync.dma_start(
                out=xf_sb[:, bc * KCI : (bc + 1) * KCI],
                in_=xf_d[:, bc * KCI : (bc + 1) * KCI],
            )
        em_sb = singles.tile([128, 128], f32, name="em_sb")
        nc.sync.dma_start(out=em_sb, in_=em_d[:])
        bij_sb = singles.tile([128, T * NUM_U], f32, name="bij_sb")

        def mm1(rhs_groups, scale):
            """s_partial[b,(s,u)] = XT.T @ rhs, scaled; -> [128, 2*US] f32."""
            cc_sb = work.tile([128, 2 * US], f16, name="cc_sb")
            ps = [psum_s.tile([128, US], f32, name=f"s_ps{bc}") for bc in range(2)]
            for t in range(T):
                rhs = rhs_groups[t // CPG]
                tl = t % CPG
                for bc in range(2):
                    nc.tensor.matmul(
                        ps[bc],
                        lhsT=xt_sb[:, t * B + bc * 128 : t * B + bc * 128 + 128],
                        rhs=rhs[:, tl * US : (tl + 1) * US],
                        start=(t == 0),
                        stop=(t == T - 1),
                    )
            for bc in range(2):
                nc.scalar.activation(
                    out=cc_sb[:, bc * US : (bc + 1) * US],
                    in_=ps[bc],
                    func=AF.Copy,
                    scale=float(scale),
                )
            return cc_sb

        def allreduce(cc_sb):
            cc_in = dram.tile([128, 2 * US], f16, name="cc_in")
            cc_out = dram.tile([128, 2 * US], f16, name="cc_out")
            nc.gpsimd.dma_start(out=cc_in, in_=cc_sb)
            if bypass_cc:
                nc.gpsimd.dma_start(out=cc_out, in_=cc_in)
            else:
                nc.gpsimd.collective_compute(
                    "AllReduce",
                    ALU.add,
                    replica_groups=[list(range(NCORES))],
                    ins=[cc_in.opt()],
                    outs=[cc_out.opt()],
                )
            s_sb = work.tile([128, 2 * US], f16, name="s_sb")
            nc.gpsimd.dma_start(out=s_sb, in_=cc_out)
            return s_sb

        def squash(s_sb, want_bf):
            """v = |s|^2/(1+|s|^2) * s/|s|, norm over u per (b, s')."""
            sq = work.tile([128, 2 * US], f32, name="sq")
            nc.scalar.square(out=sq, in_=s_sb)
            magsq = work.tile([128, 2 * S], f32, name="magsq")
            nc.vector.reduce_sum(
                out=magsq,
                in_=sq.rearrange("p (bc s u) -> p bc s u", bc=2, s=S),
                axis=AX.X,
            )
            mag = work.tile([128, 2 * S], f32, name="mag")
            nc.scalar.sqrt(out=mag, in_=magsq)
            den = work.tile([128, 2 * S], f32, name="den")
            nc.vector.scalar_tensor_tensor(
                out=den, in0=magsq, scalar=1.0, in1=mag, op0=ALU.add, op1=ALU.mult
            )
            rden = work.tile([128, 2 * S], f32, name="rden")
            nc.vector.reciprocal(out=rden, in_=den)
            fct = work.tile([128, 2 * S], f32, name="fct")
            nc.vector.tensor_mul(out=fct, in0=magsq, in1=rden)
            v_sb = work.tile([128, 2 * US], f32, name="v_sb")
            nc.vector.tensor_mul(
                out=v_sb.rearrange("p (bc s u) -> p bc s u", bc=2, s=S),
                in0=s_sb.rearrange("p (bc s u) -> p bc s u", bc=2, s=S),
                in1=fct.rearrange("p (bc s) -> p bc s", bc=2)
                .unsqueeze(3)
                .broadcast_to([128, 2, S, NUM_U]),
            )
            if not want_bf:
                return v_sb, None
            v_bf = work.tile([128, 2 * US], f16, name="v_bf")
            nc.scalar.copy(out=v_bf, in_=v_sb)
            return v_sb, v_bf

        def routing_update(v_bf, first):
            """u_vj1 -> b_ij update -> softmax; returns c_ij (f16)."""
            r_sb = work.tile([128, T * NUM_U], f32, name="r_sb")
            for g in range(G):
                m_g = big.tile([128, CPG * US], f16, name=f"m_g{g}")
                for tl in range(CPG):
                    t = g * CPG + tl
                    ps = psum_m.tile([128, US], f32, name="m_ps")
                    for bc in range(2):
                        nc.tensor.matmul(
                            ps,
                            lhsT=xf_sb[
                                :, bc * KCI + t * 128 : bc * KCI + (t + 1) * 128
                            ],
                            rhs=v_bf[:, bc * US : (bc + 1) * US],
                            start=(bc == 0),
                            stop=(bc == 1),
                        )
                    nc.scalar.copy(out=m_g[:, tl * US : (tl + 1) * US], in_=ps)
                q_g = big.tile([128, CPG * US], f16, name=f"q_g{g}")
                nc.gpsimd.tensor_mul(
                    out=q_g,
                    in0=wm_sb[:, g * CPG * US : (g + 1) * CPG * US],
                    in1=m_g,
                )
                nc.vector.reduce_sum(
                    out=r_sb[:, g * CPG * NUM_U : (g + 1) * CPG * NUM_U],
                    in_=q_g.rearrange("p (t s u) -> p t s u", t=CPG, s=S).transpose(
                        [0, 1, 3, 2]
                    ),
                    axis=AX.X,
                )
            ups = psum_u.tile([128, T * NUM_U], f32, name="u_ps")
            nc.tensor.matmul(ups, lhsT=em_sb, rhs=r_sb, start=True, stop=True)
            if first:
                nc.vector.tensor_copy(out=bij_sb, in_=ups)
            else:
                nc.vector.tensor_add(out=bij_sb, in0=bij_sb, in1=ups)
            # softmax over u (inner groups of 10)
            mx = work.tile([128, T], f32, name="mx")
            nc.vector.reduce_max(
                out=mx, in_=bij_sb.rearrange("p (t u) -> p t u", t=T), axis=AX.X
            )
            ex = work.tile([128, T * NUM_U], f32, name="ex")
            nc.vector.tensor_sub(
                out=ex.rearrange("p (t u) -> p t u", t=T),
                in0=bij_sb.rearrange("p (t u) -> p t u", t=T),
                in1=mx[:].unsqueeze(2).broadcast_to([128, T, NUM_U]),
            )
            nc.scalar.activation(out=ex, in_=ex, func=AF.Exp)
            sm = work.tile([128, T], f32, name="sm")
            nc.vector.reduce_sum(
                out=sm, in_=ex.rearrange("p (t u) -> p t u", t=T), axis=AX.X
            )
            rsm = work.tile([128, T], f32, name="rsm")
            nc.vector.reciprocal(out=rsm, in_=sm)
            cij_sb = work.tile([128, T * NUM_U], f16, name="cij_sb")
            nc.vector.tensor_mul(
                out=cij_sb.rearrange("p (t u) -> p t u", t=T),
                in0=ex.rearrange("p (t u) -> p t u", t=T),
                in1=rsm[:].unsqueeze(2).broadcast_to([128, T, NUM_U]),
            )
            return cij_sb

        def bm_build(cij_sb):
            groups = []
            for g in range(G):
                bm_g = big.tile([128, CPG * US], f16, name=f"bm_g{g}")
                nc.vector.tensor_mul(
                    out=bm_g.rearrange("p (t s u) -> p t s u", t=CPG, s=S),
                    in0=wm_sb[:, g * CPG * US : (g + 1) * CPG * US].rearrange(
                        "p (t s u) -> p t s u", t=CPG, s=S
                    ),
                    in1=cij_sb[:, g * CPG * NUM_U : (g + 1) * CPG * NUM_U]
                    .rearrange("p (t u) -> p t u", t=CPG)
                    .unsqueeze(2)
                    .broadcast_to([128, CPG, S, NUM_U]),
                )
                groups.append(bm_g)
            return groups

        wm_groups = [
            wm_sb[:, g * CPG * US : (g + 1) * CPG * US] for g in range(G)
        ]
        v_sb = None
        for it in range(NITER):
            if it == 0:
                cc = mm1(wm_groups, 1.0 / NUM_U)
            else:
                cij = routing_update(v_bf, first=(it == 1))
                cc = mm1(bm_build(cij), 1.0)
            s_sb = allreduce(cc)
            v_sb, v_bf = squash(s_sb, want_bf=(it < NITER - 1))
        nc.sync.dma_start(out=out_d[:], in_=v_sb)

    return nc


def _prep_core_inputs(x, W, core, em):
    sl = slice(core * CL, (core + 1) * CL)
    xs = np.ascontiguousarray(x[:, :, sl])  # (B, I, CL)
    ws = np.ascontiguousarray(W[0, sl])     # (CL, U, S, I)
    xt = xs.transpose(2, 1, 0).reshape(T, 128, B)
    xt = np.ascontiguousarray(xt.transpose(1, 0, 2)).reshape(128, T * B)
    xf = xs.transpose(0, 2, 1).reshape(2, 128, KCI)
    xf = np.ascontiguousarray(xf.transpose(1, 0, 2)).reshape(128, 2 * KCI)
    wm = ws.transpose(0, 3, 2, 1).reshape(T, 128, US)
    wm = np.ascontiguousarray(wm.transpose(1, 0, 2)).reshape(128, T * US)
    return {
        "xt": xt.astype(np.float16),
        "xf": xf.astype(np.float16),
        "wm": wm.astype(np.float16),
        "em": em,
    }


def prep_in_maps(x, W):
    x = np.asarray(x, dtype=np.float32)
    W = np.asarray(W, dtype=np.float32)
    em = (np.kron(np.eye(8, dtype=np.float32), np.ones((16, 16), np.float32))
          / float(B))
    return [_prep_core_inputs(x, W, core, em) for core in range(NCORES)]


def postprocess(out_core):
    """out_core [128, 640] with col = bc*320 + s*10 + u -> (B, U, S, 1)."""
    v = out_core.reshape(128, 2, S, NUM_U).transpose(1, 0, 3, 2)  # (bc,p,u,s)
    return np.ascontiguousarray(v.reshape(B, NUM_U, S)[..., None])


def get_program():
    if "nc" not in _CACHE:
        nc = _build_program()
        nc.finalize()  # runs Bacc.compile(): reg alloc + sync-wait legalization
        _CACHE["nc"] = nc
    return _CACHE["nc"]


def kernel(x, W):
    from concourse.bass_utils import run_bass_kernel_spmd

    nc = get_program()
    in_maps = prep_in_maps(x, W)
    res = run_bass_kernel_spmd(nc, in_maps, list(range(NCORES)))
    return postprocess(np.asarray(res.results[0]["out"], dtype=np.float32))
